# revision 1
# baseline (speedup 1.0000x reference)
"""2-layer GCN (PyG GCNConv x2 + sigmoid) on 8 TRN2 NeuronCores, single fused NEFF.

Sharding: dst-node ranges across the 8 cores (6250 nodes each); GCN weights
replicated; the layer-1->layer-2 halo exchange is an on-device AllGather of
each core's 6250 z'=W2^T h' values.

Design notes (cost-model driven):
- ap_gather costs max(table_cols, num_idxs)*0.833ns -> tables and gather
  chunks must be size-matched. 7 src-quarters (table=7144 cols) with 2
  ~8K-slot chunks each keeps L1 gathers slot-optimal (~0.84ns/edge).
- Edge segment sums via exact-degree ladders with layouts shared
  (max-over-core) so one SPMD program fits all cores; k=1 rows are Act
  copies, some k<=2/3 pair-adds go to GPSIMD to balance DVE; per-quarter
  partials are perm-gathered back to node order and accumulated in bf16
  (DVE 2x mode); assembly runs one quarter behind the gathers to keep
  GPSIMD saturated.
- Tables built by PE in bf16 (1 cyc/col); both dinv_dst multiplies fold
  into bf16 tensor ops; h'@W2 contracts on PE in bf16.
- Layer 2: the 8 GPSIMD 16-partition groups each own one SRC CORE RANGE so
  per-group z tables are 6256 wide (table-cost-minimal); self-loops are
  excluded from the edge stream (their term is zrow, added per chunk);
  cross-group partial sums contract on the PE via a stride-16 ones vector
  over the perm-gathered (bf16-converted) partials.
- Finalize and the last quarter's assembly are column-sliced so the
  z-row production chain into the collective stays pipelined.
"""

import sys

sys.path.insert(0, "/opt/trn_rl_repo")
import numpy as np
import ml_dtypes
from contextlib import ExitStack

from concourse import bacc, mybir
from concourse.tile import TileContext
from concourse.bass_utils import run_bass_kernel_spmd

MEASURE = False
LAST_SIM_NS = None

N = 50000
E = 800000
F = 128
P = 128
NCORES = 8
NSH = N // NCORES  # 6250
NQ = 7
QN = 7143  # nodes per quarter (last has 7142)
T = QN + 1  # 7144: [zero col, up to 7143 nodes]
NP_ = 6256  # padded per-core node count
MMCH = 512
XB = 2048


def _wrap16(idx_flat):
    n = idx_flat.shape[0]
    assert n % 16 == 0
    return np.ascontiguousarray(idx_flat.reshape(n // 16, 16).T)


def _pad16(n):
    return ((n + 15) // 16) * 16


def _concat_aranges(lens):
    if len(lens) == 0:
        return np.zeros(0, dtype=np.int64)
    total = int(lens.sum())
    out = np.ones(total, dtype=np.int64)
    ends = np.cumsum(lens)
    out[0] = 0
    out[ends[:-1]] = -(lens[:-1] - 1)
    return np.cumsum(out)


def _ladder_layout(kap_by_unit, n_chunks_cap, kdesc=False):
    """kap_by_unit: [n_units, n_nodes]. Shared exact-k ladder with row-aligned
    chunks, big k first (heavy reduces overlap the next chunk's gather).
    Returns (descr[(chunk, off, n_rows, k, col)], cols, kbase, chunk_sizes)."""
    kmax = int(kap_by_unit.max())
    budgets = {}
    for k in range(1, kmax + 1):
        nk = int((kap_by_unit == k).sum(axis=1).max())
        if nk > 0:
            budgets[k] = nk
    raw = sum(k * nk for k, nk in budgets.items())
    cap = raw + 64 if n_chunks_cap is None else (raw + n_chunks_cap - 1) // n_chunks_cap + 48
    descr, kbase = [], {}
    col = 1
    ch, off = 0, 0
    for k in sorted(budgets, reverse=kdesc):
        nk = budgets[k]
        kbase[k] = col
        left = nk
        while left > 0:
            fit = min(left, (cap - off) // k)
            if fit == 0:
                ch += 1
                off = 0
                fit = min(left, cap // k)
            descr.append((ch, off, fit, k, col))
            off += fit * k
            col += fit
            left -= fit
    chunk_sizes = {}
    for c, o, nr, k, _ in descr:
        chunk_sizes[c] = max(chunk_sizes.get(c, 0), o + nr * k)
    sizes = [_pad16(chunk_sizes[c]) for c in sorted(chunk_sizes)]
    return descr, col, kbase, sizes


def _pack_slots(kap, srcl_by_dst, dstl_by_dst, descr, kbase, cols, chunk_offs):
    """kap: [n_nodes] this unit's degrees; srcl/dstl: this unit's edges sorted
    by dst. Returns (slot_positions, slot_values, perm[node->accp col])."""
    nodes = np.nonzero(kap)[0]
    kn = kap[nodes]
    nd = np.lexsort((nodes, kn))
    nodes_s, kn_s = nodes[nd], kn[nd]
    rank = np.zeros(len(nodes_s), dtype=np.int64)
    colof = np.zeros(len(nodes_s), dtype=np.int64)
    for k in np.unique(kn_s):
        mk = kn_s == k
        rank[mk] = np.arange(mk.sum())
        colof[mk] = kbase[int(k)]
    node_col = colof + rank
    col2slot = np.full(cols, -1, dtype=np.int64)
    for ch, off, n_rows, k, col in descr:
        cc = np.arange(n_rows)
        col2slot[col + cc] = chunk_offs[ch] + off + cc * k
    starts = col2slot[node_col]
    eslots = np.repeat(starts, kn_s) + _concat_aranges(kn_s)
    # edge values in (k, node) order: stable sort of dst-sorted edges by k
    eo = np.argsort(kap[dstl_by_dst], kind="stable")
    ev = srcl_by_dst[eo]
    pm = np.zeros(len(kap), dtype=np.int16)
    pm[nodes_s] = node_col.astype(np.int16)
    return eslots, ev, pm


def host_prep(x, edge_index, W1, b1, W2, b2):
    src = np.concatenate([edge_index[0], np.arange(N, dtype=np.int64)]).astype(np.int32)
    dst = np.concatenate([edge_index[1], np.arange(N, dtype=np.int64)]).astype(np.int32)
    deg = np.bincount(dst, minlength=N).astype(np.float32)
    dinv = 1.0 / np.sqrt(np.maximum(deg, 1e-12))
    dinv[deg <= 0] = 0.0

    # Node -> table position. Stratified round-robin: nodes with identical
    # per-core in-degree vectors spread evenly over quarters, which tightens
    # the shared (max-over-core) ladder budgets vs a random permutation.
    degc = np.zeros((N, NCORES), dtype=np.int32)
    dst_t = np.concatenate([edge_index[1], np.arange(N, dtype=np.int64)])
    src_t = np.concatenate([edge_index[0], np.arange(N, dtype=np.int64)])
    np.add.at(degc, (src_t, dst_t // NSH), 1)
    okey = np.lexsort(tuple(degc[:, c] for c in range(NCORES)))
    rank = np.empty(N, dtype=np.int64)
    rank[okey] = np.arange(N)
    psrc = (rank % NQ) * QN + rank // NQ  # node -> table position
    assert psrc.max() < NQ * QN
    pinv = np.argsort(psrc)

    xtp = (x * dinv[:, None]).T.astype(np.float32)[:, pinv]  # [128, N] pos order
    xt = np.zeros((P, NQ * T), dtype=ml_dtypes.bfloat16)
    for q in range(NQ):
        qn = min(QN, N - q * QN)
        xt[:, q * T + 1 : q * T + 1 + qn] = xtp[:, q * QN : q * QN + qn].astype(
            ml_dtypes.bfloat16
        )

    core = dst // NSH
    dstl = (dst % NSH).astype(np.int64)
    pos = psrc[src]
    quarter = pos // QN
    srcl = (pos % QN).astype(np.int64) + 1

    flat = (core.astype(np.int64) * NQ + quarter) * NSH + dstl
    kap = np.bincount(flat, minlength=NCORES * NQ * NSH).reshape(NCORES, NQ, NSH)

    layouts = []
    for q in range(NQ):
        # last quarter: big-k first so its trailing chunk is reduce-light and
        # the final perm-gather (and the finalize chain) starts sooner
        descr, cols, kbase, sizes = _ladder_layout(kap[:, q, :], 2, kdesc=(q >= NQ - 3))
        offs = np.concatenate([[0], np.cumsum(sizes)]).astype(np.int64)
        layouts.append((descr, cols, kbase, sizes, offs))
    SQ = [int(l[4][-1]) for l in layouts]
    PQ = _pad16(max(l[1] for l in layouts))
    G0 = max(max(l[3]) for l in layouts)

    order = np.lexsort((dstl, quarter, core))
    so, do_, qo, co = srcl[order], dstl[order], quarter[order], core[order]

    # combined per-quarter index stream: [SQ[q] slot idxs | NP_ perm idxs]
    qoff = np.concatenate([[0], np.cumsum([s + NP_ for s in SQ])]).astype(np.int64)
    qbase = np.concatenate([[0], np.cumsum(SQ)]).astype(np.int64)
    eidx = np.zeros((NCORES, int(qoff[-1])), dtype=np.int16)
    for c in range(NCORES):
        mc = co == c
        for q in range(NQ):
            m = mc & (qo == q)
            descr, cols, kbase, sizes, offs = layouts[q]
            eslots, ev, pm = _pack_slots(
                kap[c, q], so[m], do_[m], descr, kbase, cols, offs
            )
            eidx[c, qoff[q] + eslots] = ev.astype(np.int16)
            eidx[c, qoff[q] + SQ[q] : qoff[q] + SQ[q] + NSH] = pm

    eidx_w = np.zeros((NCORES, P, int(qoff[-1]) // 16), dtype=np.int16)
    for c in range(NCORES):
        eidx_w[c] = np.tile(_wrap16(eidx[c]), (8, 1))

    dinvb = np.zeros((NCORES, P, NP_), dtype=ml_dtypes.bfloat16)
    dinvrow = np.zeros((NCORES, 1, NP_), dtype=np.float32)
    for c in range(NCORES):
        dv = dinv[c * NSH : (c + 1) * NSH]
        dinvb[c, :, :NSH] = np.tile(dv.astype(ml_dtypes.bfloat16)[None, :], (P, 1))
        dinvrow[c, 0, :NSH] = dv

    meta = dict(layouts=layouts, SQ=SQ, PQ=PQ, G0=G0, qbase=qbase, qoff=qoff)
    k1_inputs = []
    for c in range(NCORES):
        k1_inputs.append(
            {
                "xt": xt,
                "w1": np.asarray(W1, dtype=ml_dtypes.bfloat16),
                "b1": np.asarray(b1, dtype=np.float32).reshape(P, 1),
                "w2": np.asarray(W2, dtype=ml_dtypes.bfloat16).reshape(P, 1),
                "eidx": np.ascontiguousarray(eidx_w[c]),
                "dinvb": np.ascontiguousarray(dinvb[c]),
                "dinvrow": np.ascontiguousarray(dinvrow[c]),
            }
        )
    return k1_inputs, meta, (src, dst, dinv)


def host_prep_k2(src, dst):
    """Layer 2: 8 GPSIMD groups = 8 src core ranges; self-loops excluded."""
    m = src != dst
    s2, d2 = src[m].astype(np.int64), dst[m].astype(np.int64)
    c2 = d2 // NSH
    g2 = s2 // NSH
    dstl = d2 % NSH
    srcl = s2 % NSH + 1

    flat = (c2 * NCORES + g2) * NSH + dstl
    kap2 = np.bincount(flat, minlength=NCORES * NCORES * NSH).reshape(
        NCORES * NCORES, NSH
    )
    descr2, cols2, kbase2, sizes2 = _ladder_layout(kap2, 2, kdesc=True)
    offs2 = np.concatenate([[0], np.cumsum(sizes2)]).astype(np.int64)
    slots2 = int(offs2[-1])
    P2 = _pad16(cols2)

    order = np.lexsort((dstl, g2, c2))
    so, do_, go, co = srcl[order], dstl[order], g2[order], c2[order]
    eidx2 = np.zeros((NCORES, NCORES, slots2), dtype=np.int16)
    perm2 = np.zeros((NCORES, NCORES, NP_), dtype=np.int16)
    for c in range(NCORES):
        mc = co == c
        for g in range(NCORES):
            mm = mc & (go == g)
            eslots, ev, pm = _pack_slots(
                kap2[c * NCORES + g], so[mm], do_[mm], descr2, kbase2, cols2, offs2
            )
            eidx2[c, g, eslots] = ev.astype(np.int16)
            perm2[c, g, :NSH] = pm

    eidx2_w = np.zeros((NCORES, P, slots2 // 16), dtype=np.int16)
    perm2_w = np.zeros((NCORES, P, NP_ // 16), dtype=np.int16)
    for c in range(NCORES):
        for g in range(NCORES):
            eidx2_w[c, g * 16 : (g + 1) * 16] = _wrap16(eidx2[c, g])
            perm2_w[c, g * 16 : (g + 1) * 16] = _wrap16(perm2[c, g])

    svec = np.zeros((P, 1), dtype=ml_dtypes.bfloat16)
    svec[0:P:16, 0] = 1.0  # sum the 8 group-partial rows

    meta2 = dict(descr2=descr2, P2=P2, slots2=slots2, sizes2=sizes2, offs2=offs2)
    k2_inputs = []
    for c in range(NCORES):
        k2_inputs.append(
            {
                "eidx2": np.ascontiguousarray(eidx2_w[c]),
                "perm2": np.ascontiguousarray(perm2_w[c]),
                "svec": svec,
            }
        )
    return k2_inputs, meta2


def build_fused(meta, meta2, b2val):
    layouts, SQ, PQ, G0, qbase, qoff = (
        meta["layouts"],
        meta["SQ"],
        meta["PQ"],
        meta["G0"],
        meta["qbase"],
        meta["qoff"],
    )
    ITW = (max(SQ) + NP_) // 16  # combined per-quarter idx tile width
    descr2, P2, slots2 = meta2["descr2"], meta2["P2"], meta2["slots2"]
    sizes2, offs2 = meta2["sizes2"], meta2["offs2"]
    G2 = max(sizes2)

    nc = bacc.Bacc(None, target_bir_lowering=False)
    f32, f32r, bf16, i16 = (
        mybir.dt.float32,
        mybir.dt.float32r,
        mybir.dt.bfloat16,
        mybir.dt.int16,
    )

    xt_d = nc.dram_tensor("xt", [P, NQ * T], bf16, kind="ExternalInput")
    w1_d = nc.dram_tensor("w1", [P, P], bf16, kind="ExternalInput")
    b1_d = nc.dram_tensor("b1", [P, 1], f32, kind="ExternalInput")
    w2_d = nc.dram_tensor("w2", [P, 1], bf16, kind="ExternalInput")
    eidx_d = nc.dram_tensor("eidx", [P, int(qoff[-1]) // 16], i16, kind="ExternalInput")
    dinvb_d = nc.dram_tensor("dinvb", [P, NP_], bf16, kind="ExternalInput")
    dinvrow_d = nc.dram_tensor("dinvrow", [1, NP_], f32, kind="ExternalInput")
    eidx2_d = nc.dram_tensor("eidx2", [P, slots2 // 16], i16, kind="ExternalInput")
    perm2_d = nc.dram_tensor("perm2", [P, NP_ // 16], i16, kind="ExternalInput")
    svec_d = nc.dram_tensor("svec", [P, 1], bf16, kind="ExternalInput")
    out_d = nc.dram_tensor("out", [1, NP_], f32, kind="ExternalOutput")

    zin = nc.dram_tensor("zin_cc", [1, NSH], f32, kind="Internal")
    zall = nc.dram_tensor(
        "zall_cc", [NCORES, NSH], f32, kind="Internal", addr_space="Shared"
    )

    Copy = mybir.ActivationFunctionType.Copy
    Sigmoid = mybir.ActivationFunctionType.Sigmoid
    ADD = mybir.AluOpType.add

    with ExitStack() as ctx:
        tc = ctx.enter_context(TileContext(nc))
        cpool = ctx.enter_context(tc.tile_pool(name="cpool", bufs=1))
        w1 = cpool.tile([P, P], bf16)
        b1 = cpool.tile([P, 1], f32)
        w2 = cpool.tile([P, 1], bf16)
        acc = cpool.tile([P, NP_], bf16)
        warm = cpool.tile([1, 16], f32)
        nc.sync.dma_start(out=w1[:], in_=w1_d[:])
        nc.sync.dma_start(out=b1[:], in_=b1_d[:])
        nc.sync.dma_start(out=w2[:], in_=w2_d[:])
        # preload the sigmoid activation table off the critical path
        nc.vector.memset(warm[:], 0.0)
        nc.scalar.activation(warm[:], warm[:], Sigmoid, bias=0.0)

        with (
            tc.tile_pool(name="tabs", bufs=2) as tabs,
            tc.tile_pool(name="xpool", bufs=2) as xpool,
            tc.tile_pool(name="gpool", bufs=2) as gpool,
            tc.tile_pool(name="tpool", bufs=1) as tpool,
            tc.tile_pool(name="accpool", bufs=2) as accpool,
            tc.tile_pool(name="epool", bufs=2) as epool,
            tc.tile_pool(name="pspool", bufs=2, space="PSUM") as pspool,
        ):
            accps = {}

            def assemble(q):
                # perm-gather quarter q's partials to node order and fold
                # into acc (emitted one quarter late to keep Pool saturated)
                accp_q, it_q = accps.pop(q)
                p0 = SQ[q] // 16
                tt = gpool.tile([P, G0], f32, tag="g")
                nc.gpsimd.ap_gather(
                    tt[:, :NP_],
                    accp_q[:],
                    it_q[:, p0 : p0 + NP_ // 16],
                    channels=P,
                    num_elems=PQ,
                    d=1,
                    num_idxs=NP_,
                )
                if q == 0:
                    nc.scalar.activation(acc[:], tt[:, :NP_], Copy)
                elif q < NQ - 1:
                    tb = tpool.tile([P, NP_], bf16, tag="tb")
                    nc.scalar.activation(tb[:], tt[:, :NP_], Copy)
                    nc.vector.tensor_add(acc[:], acc[:], tb[:])
                else:
                    # last quarter: slice so finalize can start per-slice
                    tb = tpool.tile([P, NP_], bf16, tag="tb")
                    for s0 in range(0, NP_, 1564):
                        sl = slice(s0, s0 + 1564)
                        nc.scalar.activation(tb[:, sl], tt[:, sl], Copy)
                        nc.vector.tensor_add(acc[:, sl], acc[:, sl], tb[:, sl])

            for q in range(NQ):
                descr, cols, kbase, sizes, offs = layouts[q]
                tab = tabs.tile([P, T], f32, tag="tab")
                chunks = []
                x0 = sum(chunks)
                while x0 < T:
                    chunks.append(min(XB, T - x0))
                    x0 += chunks[-1]
                x0 = 0
                for xw in chunks:
                    xc = xpool.tile([P, XB], bf16, tag="x")
                    nc.sync.dma_start(
                        out=xc[:, :xw], in_=xt_d[:, q * T + x0 : q * T + x0 + xw]
                    )
                    ps = pspool.tile([P, XB], f32, tag="ps")
                    for m0 in range(0, xw, MMCH):
                        mw = min(MMCH, xw - m0)
                        nc.tensor.matmul(
                            ps[:, m0 : m0 + mw],
                            w1[:],
                            xc[:, m0 : m0 + mw],
                            start=True,
                            stop=True,
                        )
                    nc.scalar.activation(tab[:, x0 : x0 + xw], ps[:, :xw], Copy)
                    x0 += xw
                accp = accpool.tile([P, PQ], f32, tag="accp")
                it = epool.tile([P, ITW], i16, tag="it")
                qw = (SQ[q] + NP_) // 16
                i0 = int(qoff[q]) // 16
                nc.sync.dma_start(out=it[:, :qw], in_=eidx_d[:, i0 : i0 + qw])
                accps[q] = (accp, it)
                nc.vector.memset(accp[:, 0:1], 0.0)
                by_chunk = {}
                for d_ in descr:
                    by_chunk.setdefault(d_[0], []).append(d_)
                for ci, ch in enumerate(sorted(by_chunk)):
                    sz = sizes[ch]
                    c0 = int(offs[ch]) // 16
                    g = gpool.tile([P, G0], f32, tag="g")
                    nc.gpsimd.ap_gather(
                        g[:, :sz],
                        tab[:],
                        it[:, c0 : c0 + sz // 16],
                        channels=P,
                        num_elems=T,
                        d=1,
                        num_idxs=sz,
                    )
                    if ci == 0 and q > 0:
                        assemble(q - 1)
                    for _, off, n_rows, k, col in by_chunk[ch]:
                        if k == 1 and ci == 0:
                            # Act handles chunk-0 k=1 rows; later chunks go to
                            # DVE so Act isn't blocked ahead of next tab build
                            nc.scalar.activation(
                                accp[:, col : col + n_rows],
                                g[:, off : off + n_rows],
                                Copy,
                            )
                        elif k == 1:
                            nc.vector.tensor_copy(
                                accp[:, col : col + n_rows], g[:, off : off + n_rows]
                            )
                        elif k == 2 and n_rows >= 48:
                            # rebalance: ~1/3 of pair-adds on GPSIMD
                            nh = _pad16(n_rows * 2 // 3)
                            pr = g[:, off : off + 2 * nh].rearrange(
                                "p (a b) -> p a b", a=nh, b=2
                            )
                            nc.gpsimd.tensor_add(
                                accp[:, col : col + nh], pr[:, :, 0], pr[:, :, 1]
                            )
                            nc.vector.tensor_reduce(
                                accp[:, col + nh : col + n_rows],
                                g[:, off + 2 * nh : off + 2 * n_rows].rearrange(
                                    "p (a b) -> p a b", a=n_rows - nh, b=2
                                ),
                                axis=mybir.AxisListType.X,
                                op=ADD,
                            )
                        else:
                            nc.vector.tensor_reduce(
                                accp[:, col : col + n_rows],
                                g[:, off : off + n_rows * k].rearrange(
                                    "p (a b) -> p a b", a=n_rows, b=k
                                ),
                                axis=mybir.AxisListType.X,
                                op=ADD,
                            )
                if q == NQ - 1:
                    assemble(q)

        # finalize (4 column slices): h' = dinv*sigmoid(dinv*acc+b1); z=W2^T h'
        with (
            tc.tile_pool(name="fin", bufs=1) as fin,
            tc.tile_pool(name="zps", bufs=2, space="PSUM") as zps,
        ):
            zrow = fin.tile([1, NP_], f32)
            dinvrow2 = fin.tile([1, NP_], f32)
            with tc.tile_pool(name="finb", bufs=1) as finb:
                dinvb = finb.tile([P, NP_], bf16)
                nc.sync.dma_start(out=dinvb[:], in_=dinvb_d[:])
                nc.sync.dma_start(out=dinvrow2[:], in_=dinvrow_d[:])
                bounds = [0, 1536, 3072, 4608, NP_]
                for si in range(4):
                    sl = slice(bounds[si], bounds[si + 1])
                    nc.vector.tensor_mul(acc[:, sl], acc[:, sl], dinvb[:, sl])
                    nc.scalar.activation(
                        acc[:, sl], acc[:, sl], Sigmoid, bias=b1[:, 0:1]
                    )
                    nc.vector.tensor_mul(acc[:, sl], acc[:, sl], dinvb[:, sl])
                    for m0 in range(bounds[si], bounds[si + 1], MMCH):
                        mw = min(MMCH, bounds[si + 1] - m0)
                        ps = zps.tile([1, MMCH], f32, tag="zp")
                        nc.tensor.matmul(
                            ps[:, :mw],
                            w2[:],
                            acc[:, m0 : m0 + mw],
                            start=True,
                            stop=True,
                        )
                        nc.scalar.activation(zrow[:, m0 : m0 + mw], ps[:, :mw], Copy)
                nc.sync.dma_start(out=zin[:, : NSH // 2], in_=zrow[:, : NSH // 2])
                nc.sync.dma_start(out=zin[:, NSH // 2 :], in_=zrow[:, NSH // 2 : NSH])

            nc.gpsimd.collective_compute(
                "AllGather",
                mybir.AluOpType.bypass,
                replica_groups=[list(range(NCORES))],
                ins=[zin[:].opt()],
                outs=[zall[:].opt()],
            )

            # ---- layer 2 ----
            with (
                tc.tile_pool(name="k2pool", bufs=1) as pool2,
                tc.tile_pool(name="zps2", bufs=2, space="PSUM") as zps2,
            ):
                eidx2 = pool2.tile([P, slots2 // 16], i16)
                perm2 = pool2.tile([P, NP_ // 16], i16)
                accp2 = pool2.tile([P, P2], f32)
                svec = pool2.tile([P, 1], bf16)
                zfin = pool2.tile([1, NP_], f32)
                nc.sync.dma_start(out=eidx2[:], in_=eidx2_d[:])
                nc.sync.dma_start(out=perm2[:], in_=perm2_d[:])
                nc.sync.dma_start(out=svec[:], in_=svec_d[:])
                nc.vector.memset(accp2[:, 0:1], 0.0)
                with tc.tile_pool(name="ztpool", bufs=1) as ztpool, tc.tile_pool(
                    name="g2pool", bufs=2
                ) as g2pool:
                    zt = ztpool.tile([P, NP_], f32)
                    nc.vector.memset(zt[:, 0:1], 0.0)
                    nc.sync.dma_start(out=zt[0:P:16, 1 : 1 + NSH], in_=zall[:, :])
                    by_chunk2 = {}
                    for d_ in descr2:
                        by_chunk2.setdefault(d_[0], []).append(d_)
                    for ch in sorted(by_chunk2):
                        sz = sizes2[ch]
                        g2 = g2pool.tile([P, G2], f32, tag="g2")
                        i0 = int(offs2[ch]) // 16
                        nc.gpsimd.ap_gather(
                            g2[:, :sz],
                            zt[:],
                            eidx2[:, i0 : i0 + sz // 16],
                            channels=P,
                            num_elems=NP_,
                            d=1,
                            num_idxs=sz,
                        )
                        for _, off, n_rows, k, col in by_chunk2[ch]:
                            if k == 1:
                                nc.scalar.activation(
                                    accp2[:, col : col + n_rows],
                                    g2[:, off : off + n_rows],
                                    Copy,
                                )
                            elif k == 2:
                                # pair-adds on GPSIMD (Pool idles post-gather)
                                pairs = g2[:, off : off + 2 * n_rows].rearrange(
                                    "p (a b) -> p a b", a=n_rows, b=2
                                )
                                nc.gpsimd.tensor_add(
                                    accp2[:, col : col + n_rows],
                                    pairs[:, :, 0],
                                    pairs[:, :, 1],
                                )
                            elif k == 3:
                                trip = g2[:, off : off + 3 * n_rows].rearrange(
                                    "p (a b) -> p a b", a=n_rows, b=3
                                )
                                nc.gpsimd.tensor_add(
                                    accp2[:, col : col + n_rows],
                                    trip[:, :, 0],
                                    trip[:, :, 1],
                                )
                                nc.vector.tensor_add(
                                    accp2[:, col : col + n_rows],
                                    accp2[:, col : col + n_rows],
                                    trip[:, :, 2],
                                )
                            else:
                                nc.vector.tensor_reduce(
                                    accp2[:, col : col + n_rows],
                                    g2[:, off : off + n_rows * k].rearrange(
                                        "p (a b) -> p a b", a=n_rows, b=k
                                    ),
                                    axis=mybir.AxisListType.X,
                                    op=ADD,
                                )
                with tc.tile_pool(name="gp2", bufs=1) as gp2pool:
                    g2p = gp2pool.tile([P, NP_], f32)
                    g2pb = gp2pool.tile([P, NP_], bf16)
                    nc.gpsimd.ap_gather(
                        g2p[:],
                        accp2[:],
                        perm2[:],
                        channels=P,
                        num_elems=P2,
                        d=1,
                        num_idxs=NP_,
                    )
                    HB = NP_ // 2
                    for s0 in (0, HB):
                        nc.scalar.activation(
                            g2pb[:, s0 : s0 + HB], g2p[:, s0 : s0 + HB], Copy
                        )
                        for m0 in range(s0, s0 + HB, MMCH):
                            mw = min(MMCH, s0 + HB - m0)
                            ps = zps2.tile([1, MMCH], f32, tag="zp2")
                            nc.tensor.matmul(
                                ps[:, :mw],
                                svec[:],
                                g2pb[:, m0 : m0 + mw],
                                start=True,
                                stop=True,
                            )
                            nc.scalar.activation(zfin[:, m0 : m0 + mw], ps[:, :mw], Copy)
                            # += self-loop term, then * dinv_dst
                            nc.vector.tensor_add(
                                zfin[:, m0 : m0 + mw],
                                zfin[:, m0 : m0 + mw],
                                zrow[:, m0 : m0 + mw],
                            )
                            nc.vector.tensor_mul(
                                zfin[:, m0 : m0 + mw],
                                zfin[:, m0 : m0 + mw],
                                dinvrow2[:, m0 : m0 + mw],
                            )
                        nc.scalar.activation(
                            zfin[:, s0 : s0 + HB],
                            zfin[:, s0 : s0 + HB],
                            Sigmoid,
                            bias=float(b2val),
                        )
                        nc.sync.dma_start(
                            out=out_d[:, s0 : s0 + HB], in_=zfin[:, s0 : s0 + HB]
                        )
    nc.finalize()
    return nc


def _sim_ns(nc):
    from concourse import bass_interp

    sim = bass_interp.CoreSim(nc, no_exec=True, publish_trace=False)
    sim.simulate()
    return int(sim.time)


def kernel(x, edge_index, W1, b1, W2, b2):
    global LAST_SIM_NS
    x = np.asarray(x, dtype=np.float32)
    edge_index = np.asarray(edge_index)
    k1_inputs, meta, (src, dst, dinv) = host_prep(x, edge_index, W1, b1, W2, b2)
    k2_inputs, meta2 = host_prep_k2(src, dst)
    b2val = float(np.asarray(b2, dtype=np.float32).reshape(-1)[0])
    nc = build_fused(meta, meta2, b2val)
    if MEASURE:
        LAST_SIM_NS = _sim_ns(nc)
    in_maps = [dict(k1_inputs[c], **k2_inputs[c]) for c in range(NCORES)]
    res = run_bass_kernel_spmd(nc, in_maps, list(range(NCORES)))
    out = np.zeros((N, 1), dtype=np.float32)
    for c in range(NCORES):
        out[c * NSH : (c + 1) * NSH, 0] = res.results[c]["out"][0, :NSH]
    return out



# revision 15
# speedup vs baseline: 1.1059x; 1.1059x over previous
"""2-layer GCN (PyG GCNConv x2 + sigmoid) on 8 TRN2 NeuronCores, single fused NEFF.

Sharding: dst-node ranges across the 8 cores for layer 1 (6250 nodes each);
GCN weights replicated; layer 2 is computed SRC-side (each core aggregates
messages from its own z values into per-dst-core partials) and a single
ReduceScatter(add) of the partials replaces the AllGather + remote-z pass.

Design notes (cost-model driven):
- ap_gather costs max(table_cols, num_idxs)*0.833ns -> tables and gather
  chunks must be size-matched. 7 src-quarters (table=7144 cols) with 2
  ~8K-slot chunks each keeps L1 gathers slot-optimal.
- Edge segment sums via exact-degree ladders with layouts shared
  (max-over-core); k=1 rows are Act copies; per-quarter partials are
  perm-gathered back to node order and accumulated in bf16 (DVE 2x mode);
  assembly runs one quarter behind the gathers to keep GPSIMD saturated.
  All pair/triple adds stay on DVE - GPSIMD does gathers only.
- z = dinv*(W2^T h) is produced BROADCAST across all 128 partitions by
  replicating W2 into a [128,128] stationary (same PE cost as one row), so
  the layer-2 gather table needs no broadcast step.
- Layer 2 src-side: the 8 GPSIMD 16-partition groups each own one DST core
  range, gathering from the core's own z table (no collective needed before
  the edge work). Per-group partials go to DRAM as 8 single-row stores
  (1-D DRAM APs are cheap) and one ReduceScatter(add) returns this core's
  6250 summed values.
- Final sigmooid runs on a [125, 50] spread layout: DMA of [1,6250] into
  one partition is charged per-partition bytes, spreading across 125
  partitions makes the post-collective tail ~1us.
- Self-loops' layer-2 term (dinv_d * z_d) is added via a flat store of the
  z row + spread reload, keeping them out of the edge ladders.
"""

import sys

sys.path.insert(0, "/opt/trn_rl_repo")
import numpy as np
import ml_dtypes
from contextlib import ExitStack

from concourse import bacc, mybir
from concourse.tile import TileContext
from concourse.bass_utils import run_bass_kernel_spmd

MEASURE = False
LAST_SIM_NS = None

N = 50000
E = 800000
F = 128
P = 128
NCORES = 8
NSH = N // NCORES  # 6250
NQ = 7
QN = 7143  # nodes per quarter (last has 7142)
T = QN + 1  # 7144: [zero col, up to 7143 nodes]
NP_ = 6256  # padded per-core node count
MMCH = 512
XB = 2048
SPP = 125  # spread layout partitions (125*50 = 6250)
SPW = 50


def _wrap16(idx_flat):
    n = idx_flat.shape[0]
    assert n % 16 == 0
    return np.ascontiguousarray(idx_flat.reshape(n // 16, 16).T)


def _pad16(n):
    return ((n + 15) // 16) * 16


def _concat_aranges(lens):
    if len(lens) == 0:
        return np.zeros(0, dtype=np.int64)
    total = int(lens.sum())
    out = np.ones(total, dtype=np.int64)
    ends = np.cumsum(lens)
    out[0] = 0
    out[ends[:-1]] = -(lens[:-1] - 1)
    return np.cumsum(out)


def _ladder_layout(kap_by_unit, n_chunks_cap, kdesc=False):
    """kap_by_unit: [n_units, n_nodes]. Shared exact-k ladder with row-aligned
    chunks, big k first (heavy reduces overlap the next chunk's gather).
    Returns (descr[(chunk, off, n_rows, k, col)], cols, kbase, chunk_sizes)."""
    kmax = int(kap_by_unit.max())
    budgets = {}
    for k in range(1, kmax + 1):
        nk = int((kap_by_unit == k).sum(axis=1).max())
        if nk > 0:
            budgets[k] = nk
    raw = sum(k * nk for k, nk in budgets.items())
    cap = raw + 64 if n_chunks_cap is None else (raw + n_chunks_cap - 1) // n_chunks_cap + 48
    descr, kbase = [], {}
    col = 1
    ch, off = 0, 0
    if kdesc == "volume":
        korder = sorted(budgets, key=lambda k: -k * budgets[k])
    else:
        korder = sorted(budgets, reverse=bool(kdesc))
    for k in korder:
        nk = budgets[k]
        kbase[k] = col
        left = nk
        while left > 0:
            fit = min(left, (cap - off) // k)
            if fit == 0:
                ch += 1
                off = 0
                fit = min(left, cap // k)
            descr.append((ch, off, fit, k, col))
            off += fit * k
            col += fit
            left -= fit
    chunk_sizes = {}
    for c, o, nr, k, _ in descr:
        chunk_sizes[c] = max(chunk_sizes.get(c, 0), o + nr * k)
    sizes = [_pad16(chunk_sizes[c]) for c in sorted(chunk_sizes)]
    return descr, col, kbase, sizes


def _pack_slots(kap, srcl_by_dst, dstl_by_dst, descr, kbase, cols, chunk_offs):
    """kap: [n_nodes] this unit's degrees; srcl/dstl: this unit's edges sorted
    by dst. Returns (slot_positions, slot_values, perm[node->accp col])."""
    nodes = np.nonzero(kap)[0]
    kn = kap[nodes]
    nd = np.lexsort((nodes, kn))
    nodes_s, kn_s = nodes[nd], kn[nd]
    rank = np.zeros(len(nodes_s), dtype=np.int64)
    colof = np.zeros(len(nodes_s), dtype=np.int64)
    for k in np.unique(kn_s):
        mk = kn_s == k
        rank[mk] = np.arange(mk.sum())
        colof[mk] = kbase[int(k)]
    node_col = colof + rank
    col2slot = np.full(cols, -1, dtype=np.int64)
    for ch, off, n_rows, k, col in descr:
        cc = np.arange(n_rows)
        col2slot[col + cc] = chunk_offs[ch] + off + cc * k
    starts = col2slot[node_col]
    eslots = np.repeat(starts, kn_s) + _concat_aranges(kn_s)
    # edge values in (k, node) order: stable sort of dst-sorted edges by k
    eo = np.argsort(kap[dstl_by_dst], kind="stable")
    ev = srcl_by_dst[eo]
    pm = np.zeros(len(kap), dtype=np.int16)
    pm[nodes_s] = node_col.astype(np.int16)
    return eslots, ev, pm


def host_prep(x, edge_index, W1, b1, W2, b2):
    src = np.concatenate([edge_index[0], np.arange(N, dtype=np.int64)]).astype(np.int32)
    dst = np.concatenate([edge_index[1], np.arange(N, dtype=np.int64)]).astype(np.int32)
    deg = np.bincount(dst, minlength=N).astype(np.float32)
    dinv = 1.0 / np.sqrt(np.maximum(deg, 1e-12))
    dinv[deg <= 0] = 0.0

    # Node -> table position. Stratified round-robin: nodes with identical
    # per-core in-degree vectors spread evenly over quarters, which tightens
    # the shared (max-over-core) ladder budgets vs a random permutation.
    degc = np.zeros((N, NCORES), dtype=np.int32)
    dst_t = np.concatenate([edge_index[1], np.arange(N, dtype=np.int64)])
    src_t = np.concatenate([edge_index[0], np.arange(N, dtype=np.int64)])
    np.add.at(degc, (src_t, dst_t // NSH), 1)
    okey = np.lexsort(tuple(degc[:, c] for c in range(NCORES)))
    rank = np.empty(N, dtype=np.int64)
    rank[okey] = np.arange(N)
    psrc = (rank % NQ) * QN + rank // NQ  # node -> table position
    assert psrc.max() < NQ * QN
    pinv = np.argsort(psrc)

    xtp = (x * dinv[:, None]).T.astype(np.float32)[:, pinv]  # [128, N] pos order
    xt = np.zeros((P, NQ * T), dtype=ml_dtypes.bfloat16)
    for q in range(NQ):
        qn = min(QN, N - q * QN)
        xt[:, q * T + 1 : q * T + 1 + qn] = xtp[:, q * QN : q * QN + qn].astype(
            ml_dtypes.bfloat16
        )

    core = dst // NSH
    dstl = (dst % NSH).astype(np.int64)
    pos = psrc[src]
    quarter = pos // QN
    srcl = (pos % QN).astype(np.int64) + 1

    flat = (core.astype(np.int64) * NQ + quarter) * NSH + dstl
    kap = np.bincount(flat, minlength=NCORES * NQ * NSH).reshape(NCORES, NQ, NSH)

    layouts = []
    for q in range(NQ):
        # last quarter: big-k first so its trailing chunk is reduce-light and
        # the final perm-gather (and the finalize chain) starts sooner
        descr, cols, kbase, sizes = _ladder_layout(kap[:, q, :], 2, kdesc=(q >= NQ - 3))
        offs = np.concatenate([[0], np.cumsum(sizes)]).astype(np.int64)
        layouts.append((descr, cols, kbase, sizes, offs))
    SQ = [int(l[4][-1]) for l in layouts]
    PQ = _pad16(max(l[1] for l in layouts))
    G0 = max(max(l[3]) for l in layouts)

    order = np.lexsort((dstl, quarter, core))
    so, do_, qo, co = srcl[order], dstl[order], quarter[order], core[order]

    # combined per-quarter index stream: [SQ[q] slot idxs | NP_ perm idxs]
    qoff = np.concatenate([[0], np.cumsum([s + NP_ for s in SQ])]).astype(np.int64)
    qbase = np.concatenate([[0], np.cumsum(SQ)]).astype(np.int64)
    eidx = np.zeros((NCORES, int(qoff[-1])), dtype=np.int16)
    for c in range(NCORES):
        mc = co == c
        for q in range(NQ):
            m = mc & (qo == q)
            descr, cols, kbase, sizes, offs = layouts[q]
            eslots, ev, pm = _pack_slots(
                kap[c, q], so[m], do_[m], descr, kbase, cols, offs
            )
            eidx[c, qoff[q] + eslots] = ev.astype(np.int16)
            eidx[c, qoff[q] + SQ[q] : qoff[q] + SQ[q] + NSH] = pm

    eidx_w = np.zeros((NCORES, P, int(qoff[-1]) // 16), dtype=np.int16)
    for c in range(NCORES):
        eidx_w[c] = np.tile(_wrap16(eidx[c]), (8, 1))

    dinvb = np.zeros((NCORES, P, NP_), dtype=ml_dtypes.bfloat16)
    dinvsp = np.zeros((NCORES, SPP, SPW), dtype=np.float32)
    for c in range(NCORES):
        dv = dinv[c * NSH : (c + 1) * NSH]
        dinvb[c, :, :NSH] = np.tile(dv.astype(ml_dtypes.bfloat16)[None, :], (P, 1))
        dinvsp[c] = dv.reshape(SPP, SPW)

    w2rep = np.tile(np.asarray(W2, dtype=ml_dtypes.bfloat16).reshape(P, 1), (1, P))

    meta = dict(layouts=layouts, SQ=SQ, PQ=PQ, G0=G0, qbase=qbase, qoff=qoff)
    k1_inputs = []
    for c in range(NCORES):
        k1_inputs.append(
            {
                "xt": xt,
                "w1": np.asarray(W1, dtype=ml_dtypes.bfloat16),
                "b1": np.asarray(b1, dtype=np.float32).reshape(P, 1),
                "w2rep": w2rep,
                "eidx": np.ascontiguousarray(eidx_w[c]),
                "dinvb": np.ascontiguousarray(dinvb[c]),
                "dinvsp": np.ascontiguousarray(dinvsp[c]),
            }
        )
    return k1_inputs, meta, (src, dst, dinv)


def host_prep_k2(src, dst):
    """Layer 2, src-side: each core owns the edges whose SRC is in its node
    range; the 8 GPSIMD groups = 8 DST core ranges. Self-loops excluded."""
    m = src != dst
    s2, d2 = src[m].astype(np.int64), dst[m].astype(np.int64)
    c2 = s2 // NSH  # owning core (src side)
    g2 = d2 // NSH  # GPSIMD group (dst core)
    dstl = d2 % NSH
    srcl = s2 % NSH + 1  # index into own z table [zero col + 6250]

    flat = (c2 * NCORES + g2) * NSH + dstl
    kap2 = np.bincount(flat, minlength=NCORES * NCORES * NSH).reshape(
        NCORES * NCORES, NSH
    )
    descr2, cols2, kbase2, sizes2 = _ladder_layout(kap2, 2, kdesc="volume")
    offs2 = np.concatenate([[0], np.cumsum(sizes2)]).astype(np.int64)
    slots2 = int(offs2[-1])
    P2 = _pad16(cols2)

    order = np.lexsort((dstl, g2, c2))
    so, do_, go, co = srcl[order], dstl[order], g2[order], c2[order]
    eidx2 = np.zeros((NCORES, NCORES, slots2), dtype=np.int16)
    perm2 = np.zeros((NCORES, NCORES, NP_), dtype=np.int16)
    for c in range(NCORES):
        mc = co == c
        for g in range(NCORES):
            mm = mc & (go == g)
            eslots, ev, pm = _pack_slots(
                kap2[c * NCORES + g], so[mm], do_[mm], descr2, kbase2, cols2, offs2
            )
            eidx2[c, g, eslots] = ev.astype(np.int16)
            perm2[c, g, :NSH] = pm

    eidx2_w = np.zeros((NCORES, P, slots2 // 16), dtype=np.int16)
    perm2_w = np.zeros((NCORES, P, NP_ // 16), dtype=np.int16)
    for c in range(NCORES):
        for g in range(NCORES):
            eidx2_w[c, g * 16 : (g + 1) * 16] = _wrap16(eidx2[c, g])
            perm2_w[c, g * 16 : (g + 1) * 16] = _wrap16(perm2[c, g])

    meta2 = dict(descr2=descr2, P2=P2, slots2=slots2, sizes2=sizes2, offs2=offs2)
    k2_inputs = []
    for c in range(NCORES):
        k2_inputs.append(
            {
                "eidx2": np.ascontiguousarray(eidx2_w[c]),
                "perm2": np.ascontiguousarray(perm2_w[c]),
            }
        )
    return k2_inputs, meta2


def build_fused(meta, meta2, b2val):
    layouts, SQ, PQ, G0, qbase, qoff = (
        meta["layouts"],
        meta["SQ"],
        meta["PQ"],
        meta["G0"],
        meta["qbase"],
        meta["qoff"],
    )
    ITW = (max(SQ) + NP_) // 16  # combined per-quarter idx tile width
    descr2, P2, slots2 = meta2["descr2"], meta2["P2"], meta2["slots2"]
    sizes2, offs2 = meta2["sizes2"], meta2["offs2"]
    G2 = max(sizes2)

    nc = bacc.Bacc(None, target_bir_lowering=False)
    f32, bf16, i16 = (mybir.dt.float32, mybir.dt.bfloat16, mybir.dt.int16)

    xt_d = nc.dram_tensor("xt", [P, NQ * T], bf16, kind="ExternalInput")
    w1_d = nc.dram_tensor("w1", [P, P], bf16, kind="ExternalInput")
    b1_d = nc.dram_tensor("b1", [P, 1], f32, kind="ExternalInput")
    w2rep_d = nc.dram_tensor("w2rep", [P, P], bf16, kind="ExternalInput")
    eidx_d = nc.dram_tensor("eidx", [P, int(qoff[-1]) // 16], i16, kind="ExternalInput")
    dinvb_d = nc.dram_tensor("dinvb", [P, NP_], bf16, kind="ExternalInput")
    dinvsp_d = nc.dram_tensor("dinvsp", [SPP, SPW], f32, kind="ExternalInput")
    eidx2_d = nc.dram_tensor("eidx2", [P, slots2 // 16], i16, kind="ExternalInput")
    perm2_d = nc.dram_tensor("perm2", [P, NP_ // 16], i16, kind="ExternalInput")
    out_d = nc.dram_tensor("out", [SPP, SPW], f32, kind="ExternalOutput")

    zrow_d = nc.dram_tensor("zrow_cc", [1, NSH], f32, kind="Internal")
    zpin = nc.dram_tensor("zpin_cc", [NCORES, NSH], f32, kind="Internal")
    zpout = nc.dram_tensor("zpout_cc", [1, NSH], f32, kind="Internal")

    Copy = mybir.ActivationFunctionType.Copy
    Sigmoid = mybir.ActivationFunctionType.Sigmoid
    ADD = mybir.AluOpType.add

    def seg_sum(accp, g, off, n_rows, k, col):
        # k-segment sum. For small k: (k-1) strided adds - DVE is charged max
        # operand free-size, so each add costs n_rows instead of reduce's
        # k*n_rows. Large k keeps the single-instruction reduce (short dep
        # chain, small classes).
        if k > 3:
            nc.vector.tensor_reduce(
                accp[:, col : col + n_rows],
                g[:, off : off + n_rows * k].rearrange(
                    "p (a b) -> p a b", a=n_rows, b=k
                ),
                axis=mybir.AxisListType.X,
                op=ADD,
            )
            return
        lanes = g[:, off : off + n_rows * k].rearrange("p (a b) -> p a b", a=n_rows, b=k)
        nc.vector.tensor_add(
            accp[:, col : col + n_rows], lanes[:, :, 0], lanes[:, :, 1]
        )
        for j in range(2, k):
            nc.vector.tensor_add(
                accp[:, col : col + n_rows],
                accp[:, col : col + n_rows],
                lanes[:, :, j],
            )

    with ExitStack() as ctx:
        tc = ctx.enter_context(TileContext(nc))
        cpool = ctx.enter_context(tc.tile_pool(name="cpool", bufs=1))
        w1 = cpool.tile([P, P], bf16)
        b1 = cpool.tile([P, 1], f32)
        w2rep = cpool.tile([P, P], bf16)
        acc = cpool.tile([P, NP_], bf16)
        warm = cpool.tile([1, 16], f32)
        # b1/w2rep are finalize-only: loaded late so they don't delay tab 0
        # preload the sigmoid activation table off the critical path
        nc.vector.memset(warm[:], 0.0)
        nc.scalar.activation(warm[:], warm[:], Sigmoid, bias=0.0)

        with (
            tc.tile_pool(name="accpool", bufs=2) as accpool,
            tc.tile_pool(name="epool", bufs=2) as epool,
        ):
            accps = {}

            with (
                tc.tile_pool(name="tabs", bufs=2) as tabs,
                tc.tile_pool(name="xpool", bufs=2) as xpool,
                tc.tile_pool(name="gpool", bufs=2) as gpool,
                tc.tile_pool(name="tpool", bufs=1) as tpool,
                tc.tile_pool(name="pspool", bufs=2, space="PSUM") as pspool,
            ):

                def assemble(q):
                    # perm-gather quarter q's partials to node order and fold
                    # into acc (emitted one quarter late to keep Pool busy)
                    accp_q, it_q = accps.pop(q)
                    p0 = SQ[q] // 16
                    tt = gpool.tile([P, G0], f32, tag="g")
                    nc.gpsimd.ap_gather(
                        tt[:, :NP_],
                        accp_q[:],
                        it_q[:, p0 : p0 + NP_ // 16],
                        channels=P,
                        num_elems=PQ,
                        d=1,
                        num_idxs=NP_,
                    )
                    if q == 0:
                        nc.scalar.activation(acc[:], tt[:, :NP_], Copy)
                    else:
                        tb = tpool.tile([P, NP_], bf16, tag="tb")
                        nc.scalar.activation(tb[:], tt[:, :NP_], Copy)
                        nc.vector.tensor_add(acc[:], acc[:], tb[:])

                for q in range(NQ):
                    descr, cols, kbase, sizes, offs = layouts[q]
                    tab = tabs.tile([P, T], f32, tag="tab")
                    accp = accpool.tile([P, PQ], f32, tag="accp")
                    it = epool.tile([P, ITW], i16, tag="it")
                    qw = (SQ[q] + NP_) // 16
                    i0 = int(qoff[q]) // 16
                    nc.sync.dma_start(out=it[:, :qw], in_=eidx_d[:, i0 : i0 + qw])
                    nc.vector.memset(accp[:, 0:1], 0.0)
                    xbq = XB
                    chunks = []
                    x0 = sum(chunks)
                    while x0 < T:
                        chunks.append(min(xbq, T - x0))
                        x0 += chunks[-1]
                    x0 = 0
                    for xw in chunks:
                        xc = xpool.tile([P, XB], bf16, tag="x")
                        nc.sync.dma_start(
                            out=xc[:, :xw], in_=xt_d[:, q * T + x0 : q * T + x0 + xw]
                        )
                        ps = pspool.tile([P, XB], f32, tag="ps")
                        for m0 in range(0, xw, MMCH):
                            mw = min(MMCH, xw - m0)
                            nc.tensor.matmul(
                                ps[:, m0 : m0 + mw],
                                w1[:],
                                xc[:, m0 : m0 + mw],
                                start=True,
                                stop=True,
                            )
                        nc.scalar.activation(tab[:, x0 : x0 + xw], ps[:, :xw], Copy)
                        x0 += xw
                    accps[q] = (accp, it)
                    by_chunk = {}
                    for d_ in descr:
                        by_chunk.setdefault(d_[0], []).append(d_)
                    for ci, ch in enumerate(sorted(by_chunk)):
                        sz = sizes[ch]
                        c0 = int(offs[ch]) // 16
                        g = gpool.tile([P, G0], f32, tag="g")
                        nc.gpsimd.ap_gather(
                            g[:, :sz],
                            tab[:],
                            it[:, c0 : c0 + sz // 16],
                            channels=P,
                            num_elems=T,
                            d=1,
                            num_idxs=sz,
                        )
                        if ci == 1 and q > 0:
                            assemble(q - 1)
                        for _, off, n_rows, k, col in by_chunk[ch]:
                            if k == 1:
                                nc.scalar.activation(
                                    accp[:, col : col + n_rows],
                                    g[:, off : off + n_rows],
                                    Copy,
                                )
                            else:
                                seg_sum(accp, g, off, n_rows, k, col)

            # ---- tail: last assembly interleaved with finalize; then L2 ----
            with tc.tile_pool(name="fin", bufs=1) as fin:
                zt = fin.tile([P, NP_ + 16], f32)  # layer-2 gather table: z bcast
                eidx2 = fin.tile([P, slots2 // 16], i16)
                perm2 = fin.tile([P, NP_ // 16], i16)
                spd = fin.tile([SPP, SPW], f32)
                nc.vector.memset(zt[:, 0:1], 0.0)
                nc.sync.dma_start(out=eidx2[:], in_=eidx2_d[:])
                nc.sync.dma_start(out=perm2[:], in_=perm2_d[:])
                nc.sync.dma_start(out=spd[:], in_=dinvsp_d[:])

                with (
                    tc.tile_pool(name="finb", bufs=1) as finb,
                    tc.tile_pool(name="tbs", bufs=2) as tbs,
                    tc.tile_pool(name="zps", bufs=2, space="PSUM") as zps,
                ):
                    dinvb = finb.tile([P, NP_], bf16)
                    HB = NP_ // 2
                    nc.sync.dma_start(out=dinvb[:, :HB], in_=dinvb_d[:, :HB])
                    nc.sync.dma_start(out=dinvb[:, HB:], in_=dinvb_d[:, HB:])

                    def finalize_slice(a, b):
                        # h = sigmoid(dinv*acc+b1); z = dinv * (W2^T h), z
                        # broadcast across partitions via replicated W2
                        nc.vector.tensor_mul(
                            acc[:, a:b], acc[:, a:b], dinvb[:, a:b]
                        )
                        nc.scalar.activation(
                            acc[:, a:b], acc[:, a:b], Sigmoid, bias=b1[:, 0:1]
                        )
                        for m0 in range(a, b, MMCH):
                            mw = min(MMCH, b - m0)
                            ps = zps.tile([P, MMCH], f32, tag="zp")
                            nc.tensor.matmul(
                                ps[:, :mw],
                                w2rep[:],
                                acc[:, m0 : m0 + mw],
                                start=True,
                                stop=True,
                            )
                            nc.vector.tensor_mul(
                                zt[:, 1 + m0 : 1 + m0 + mw],
                                ps[:, :mw],
                                dinvb[:, m0 : m0 + mw],
                            )

                    # last quarter: perm-gather in 2 halves, assembly and
                    # finalize column-sliced so z production starts early
                    accp_q, it_q = accps.pop(NQ - 1)
                    p0 = SQ[NQ - 1] // 16
                    for h0, h1 in ((0, 3136), (3136, NP_)):
                        hw_ = h1 - h0
                        tt = finb.tile([P, 3136], f32, tag=f"tt{h0}")
                        nc.gpsimd.ap_gather(
                            tt[:, :hw_],
                            accp_q[:],
                            it_q[:, p0 + h0 // 16 : p0 + h1 // 16],
                            channels=P,
                            num_elems=PQ,
                            d=1,
                            num_idxs=hw_,
                        )
                        step = hw_ // 4
                        for s in range(4):
                            a = h0 + s * step
                            b = a + step
                            tb = tbs.tile([P, 784], bf16, tag="tbs")
                            nc.scalar.activation(
                                tb[:, : b - a], tt[:, a - h0 : b - h0], Copy
                            )
                            nc.vector.tensor_add(
                                acc[:, a:b], acc[:, a:b], tb[:, : b - a]
                            )
                            finalize_slice(a, b)

                    nc.sync.dma_start(out=zrow_d[:], in_=zt[0:1, 1 : 1 + NSH])

                # ---- layer 2 (src-side) ----
                with (
                    tc.tile_pool(name="l2a", bufs=1) as l2a,
                    tc.tile_pool(name="g2pool", bufs=2) as g2pool,
                ):
                    accp2 = l2a.tile([P, P2], f32)
                    nc.vector.memset(accp2[:, 0:1], 0.0)
                    by_chunk2 = {}
                    for d_ in descr2:
                        by_chunk2.setdefault(d_[0], []).append(d_)
                    for ch in sorted(by_chunk2):
                        sz = sizes2[ch]
                        g2 = g2pool.tile([P, G2], f32, tag="g2")
                        i0 = int(offs2[ch]) // 16
                        nc.gpsimd.ap_gather(
                            g2[:, :sz],
                            zt[:, :NP_],
                            eidx2[:, i0 : i0 + sz // 16],
                            channels=P,
                            num_elems=NP_,
                            d=1,
                            num_idxs=sz,
                        )
                        for _, off, n_rows, k, col in by_chunk2[ch]:
                            if k == 1:
                                nc.scalar.activation(
                                    accp2[:, col : col + n_rows],
                                    g2[:, off : off + n_rows],
                                    Copy,
                                )
                            else:
                                seg_sum(accp2, g2, off, n_rows, k, col)
                    g2p = l2a.tile([P, NP_], f32)
                    nc.gpsimd.ap_gather(
                        g2p[:],
                        accp2[:],
                        perm2[:],
                        channels=P,
                        num_elems=P2,
                        d=1,
                        num_idxs=NP_,
                    )
                    # per-group partial rows -> DRAM (1-D out APs are cheap);
                    # alternate DGE queues so the stores pipeline
                    queues = [nc.sync, nc.scalar]
                    for j in range(NCORES):
                        queues[j % 2].dma_start(
                            out=zpin[j : j + 1, :], in_=g2p[16 * j : 16 * j + 1, :NSH]
                        )
                    nc.gpsimd.collective_compute(
                        "ReduceScatter",
                        ADD,
                        replica_groups=[list(range(NCORES))],
                        ins=[zpin[:].opt()],
                        outs=[zpout[:].opt()],
                    )
                    # final: out = sigmoid(dinv*(rs + z_self) + b2), all in a
                    # [125, 50] spread layout to keep the tail DMAs wide
                    spr = l2a.tile([SPP, SPW], f32)
                    spz = l2a.tile([SPP, SPW], f32)
                    nc.sync.dma_start(
                        out=spz[:], in_=zrow_d[:].rearrange("a (p m) -> (a p) m", p=SPP)
                    )
                    nc.sync.dma_start(
                        out=spr[:], in_=zpout[:].rearrange("a (p m) -> (a p) m", p=SPP)
                    )
                    nc.vector.tensor_add(spr[:], spr[:], spz[:])
                    nc.vector.tensor_mul(spr[:], spr[:], spd[:])
                    nc.scalar.activation(spr[:], spr[:], Sigmoid, bias=float(b2val))
                    nc.sync.dma_start(out=out_d[:], in_=spr[:])
    nc.finalize()
    return nc


def _sim_ns(nc):
    from concourse import bass_interp

    sim = bass_interp.CoreSim(nc, no_exec=True, publish_trace=False)
    sim.simulate()
    return int(sim.time)


def kernel(x, edge_index, W1, b1, W2, b2):
    global LAST_SIM_NS
    x = np.asarray(x, dtype=np.float32)
    edge_index = np.asarray(edge_index)
    k1_inputs, meta, (src, dst, dinv) = host_prep(x, edge_index, W1, b1, W2, b2)
    k2_inputs, meta2 = host_prep_k2(src, dst)
    b2val = float(np.asarray(b2, dtype=np.float32).reshape(-1)[0])
    nc = build_fused(meta, meta2, b2val)
    if MEASURE:
        LAST_SIM_NS = _sim_ns(nc)
    in_maps = [dict(k1_inputs[c], **k2_inputs[c]) for c in range(NCORES)]
    res = run_bass_kernel_spmd(nc, in_maps, list(range(NCORES)))
    out = np.zeros((N, 1), dtype=np.float32)
    for c in range(NCORES):
        out[c * NSH : (c + 1) * NSH, 0] = res.results[c]["out"].reshape(-1)
    return out


# revision 43
# speedup vs baseline: 1.1889x; 1.0751x over previous
"""2-layer GCN (PyG GCNConv x2 + sigmoid) on 8 TRN2 NeuronCores, single fused NEFF.

Sharding: dst-node ranges across the 8 cores for layer 1 (6250 nodes each);
GCN weights replicated; layer 2 is computed SRC-side (each core aggregates
messages from its own z values into per-dst-core partials) and a single
ReduceScatter(add) of the partials replaces the AllGather + remote-z pass.

Design notes (cost-model driven):
- ap_gather costs max(table_cols, num_idxs)*0.833ns -> tables and gather
  chunks must be size-matched. 7 src-quarters (table=7144 cols) with 2
  ~8K-slot chunks each keeps L1 gathers slot-optimal.
- Edge segment sums via exact-degree ladders with layouts shared
  (max-over-core); k=1 rows are Act copies; per-quarter partials are
  perm-gathered back to node order and accumulated in bf16 (DVE 2x mode);
  assembly runs one quarter behind the gathers to keep GPSIMD saturated.
  All pair/triple adds stay on DVE - GPSIMD does gathers only.
- z = dinv*(W2^T h) is produced BROADCAST across all 128 partitions by
  replicating W2 into a [128,128] stationary (same PE cost as one row), so
  the layer-2 gather table needs no broadcast step.
- Layer 2 src-side: the 8 GPSIMD 16-partition groups each own one DST core
  range, gathering from the core's own z table (no collective needed before
  the edge work). Per-group partials go to DRAM as 8 single-row stores
  (1-D DRAM APs are cheap) and one ReduceScatter(add) returns this core's
  6250 summed values.
- Final sigmooid runs on a [125, 50] spread layout: DMA of [1,6250] into
  one partition is charged per-partition bytes, spreading across 125
  partitions makes the post-collective tail ~1us.
- Self-loops' layer-2 term (dinv_d * z_d) is added via a flat store of the
  z row + spread reload, keeping them out of the edge ladders.
"""

import sys

sys.path.insert(0, "/opt/trn_rl_repo")
import numpy as np
import ml_dtypes
from contextlib import ExitStack

from concourse import bacc, mybir
from concourse.tile import TileContext
from concourse.bass_utils import run_bass_kernel_spmd

MEASURE = False
LAST_SIM_NS = None

N = 50000
E = 800000
F = 128
P = 128
NCORES = 8
NSH = N // NCORES  # 6250
NQ = 7
QN = 7143  # nodes per quarter (last has 7142)
T = QN + 1  # 7144: [zero col, up to 7143 nodes]
NP_ = 6256  # padded per-core node count
MMCH = 512
XB = 2048
SPP = 125  # spread layout partitions (125*50 = 6250)
SPW = 50


def _wrap16(idx_flat):
    n = idx_flat.shape[0]
    assert n % 16 == 0
    return np.ascontiguousarray(idx_flat.reshape(n // 16, 16).T)


def _pad16(n):
    return ((n + 15) // 16) * 16


def _concat_aranges(lens):
    if len(lens) == 0:
        return np.zeros(0, dtype=np.int64)
    total = int(lens.sum())
    out = np.ones(total, dtype=np.int64)
    ends = np.cumsum(lens)
    out[0] = 0
    out[ends[:-1]] = -(lens[:-1] - 1)
    return np.cumsum(out)


def _ladder_layout(kap_by_unit, n_chunks_cap, kdesc=False, bias=0.30):
    """kap_by_unit: [n_units, n_nodes]. Shared exact-k ladder with row-aligned
    chunks, big k first (heavy reduces overlap the next chunk's gather).
    Returns (descr[(chunk, off, n_rows, k, col)], cols, kbase, chunk_sizes)."""
    kmax = int(kap_by_unit.max())
    budgets = {}
    for k in range(1, kmax + 1):
        nk = int((kap_by_unit == k).sum(axis=1).max())
        if nk > 0:
            budgets[k] = nk
    raw = sum(k * nk for k, nk in budgets.items())
    if kdesc == "balanced" and n_chunks_cap == 2:
        # Partition classes into 2 chunks balancing DVE drain time (chained
        # adds: (k-1)/k per slot for k<=3, 1.0 for k>3; k=1 is Act-only) while
        # keeping slot counts within cap. Chunk 1's drain overlaps the next
        # Pool op, so give it the lighter share.
        def w_of(k, nk):
            if k == 1:
                return 0.0
            if k <= 5:
                return (k - 1.0) / k * k * nk
            return float(k * nk)

        cap = raw // 2 + 48
        classes = sorted(budgets, key=lambda k: -w_of(k, budgets[k]))
        sets = {0: [], 1: []}
        wsum = {0: 0.0, 1: 0.0}
        ssum = {0: 0, 1: 0}
        for k in classes:
            sk = k * budgets[k]
            wk = w_of(k, budgets[k])
            pick = 1 if wsum[1] + wk <= (1.0 - bias) * wsum[0] else 0
            if ssum[pick] + sk > cap + 2048:
                pick = 1 - pick
            sets[pick].append(k)
            wsum[pick] += wk
            ssum[pick] += sk
        first = 0 if wsum[0] >= wsum[1] else 1  # heavier-drain set first
        korder = sorted(sets[first], reverse=True) + sorted(sets[1 - first], reverse=True)
        cap = raw // 2 + 48  # slot-even chunks; straddling class splits
    else:
        cap = raw + 64 if n_chunks_cap is None else (raw + n_chunks_cap - 1) // n_chunks_cap + 48
        if kdesc == "volume":
            korder = sorted(budgets, key=lambda k: -k * budgets[k])
        else:
            korder = sorted(budgets, reverse=bool(kdesc))
    descr, kbase = [], {}
    col = 1
    ch, off = 0, 0
    for k in korder:
        nk = budgets[k]
        kbase[k] = col
        left = nk
        while left > 0:
            fit = min(left, (cap - off) // k)
            if fit == 0:
                ch += 1
                off = 0
                fit = min(left, cap // k)
            descr.append((ch, off, fit, k, col))
            off += fit * k
            col += fit
            left -= fit
    chunk_sizes = {}
    for c, o, nr, k, _ in descr:
        chunk_sizes[c] = max(chunk_sizes.get(c, 0), o + nr * k)
    sizes = [_pad16(chunk_sizes[c]) for c in sorted(chunk_sizes)]
    return descr, col, kbase, sizes


def _pack_slots(kap, srcl_by_dst, dstl_by_dst, descr, kbase, cols, chunk_offs):
    """kap: [n_nodes] this unit's degrees; srcl/dstl: this unit's edges sorted
    by dst. Returns (slot_positions, slot_values, perm[node->accp col])."""
    nodes = np.nonzero(kap)[0]
    kn = kap[nodes]
    nd = np.lexsort((nodes, kn))
    nodes_s, kn_s = nodes[nd], kn[nd]
    rank = np.zeros(len(nodes_s), dtype=np.int64)
    colof = np.zeros(len(nodes_s), dtype=np.int64)
    for k in np.unique(kn_s):
        mk = kn_s == k
        rank[mk] = np.arange(mk.sum())
        colof[mk] = kbase[int(k)]
    node_col = colof + rank
    col2slot = np.full(cols, -1, dtype=np.int64)
    for ch, off, n_rows, k, col in descr:
        cc = np.arange(n_rows)
        col2slot[col + cc] = chunk_offs[ch] + off + cc * k
    starts = col2slot[node_col]
    eslots = np.repeat(starts, kn_s) + _concat_aranges(kn_s)
    # edge values in (k, node) order: stable sort of dst-sorted edges by k
    eo = np.argsort(kap[dstl_by_dst], kind="stable")
    ev = srcl_by_dst[eo]
    pm = np.zeros(len(kap), dtype=np.int16)
    pm[nodes_s] = node_col.astype(np.int16)
    return eslots, ev, pm


def host_prep(x, edge_index, W1, b1, W2, b2):
    src = np.concatenate([edge_index[0], np.arange(N, dtype=np.int64)]).astype(np.int32)
    dst = np.concatenate([edge_index[1], np.arange(N, dtype=np.int64)]).astype(np.int32)
    deg = np.bincount(dst, minlength=N).astype(np.float32)
    dinv = 1.0 / np.sqrt(np.maximum(deg, 1e-12))
    dinv[deg <= 0] = 0.0

    # Node -> table position. Stratified round-robin: nodes with identical
    # per-core in-degree vectors spread evenly over quarters, which tightens
    # the shared (max-over-core) ladder budgets vs a random permutation.
    degc = np.zeros((N, NCORES), dtype=np.int32)
    dst_t = np.concatenate([edge_index[1], np.arange(N, dtype=np.int64)])
    src_t = np.concatenate([edge_index[0], np.arange(N, dtype=np.int64)])
    np.add.at(degc, (src_t, dst_t // NSH), 1)
    okey = np.lexsort(tuple(degc[:, c] for c in range(NCORES)))
    rank = np.empty(N, dtype=np.int64)
    rank[okey] = np.arange(N)
    psrc = (rank % NQ) * QN + rank // NQ  # node -> table position
    assert psrc.max() < NQ * QN
    pinv = np.argsort(psrc)

    xtp = (x * dinv[:, None]).T.astype(np.float32)[:, pinv]  # [128, N] pos order
    xt = np.zeros((P, NQ * T), dtype=ml_dtypes.bfloat16)
    for q in range(NQ):
        qn = min(QN, N - q * QN)
        xt[:, q * T + 1 : q * T + 1 + qn] = xtp[:, q * QN : q * QN + qn].astype(
            ml_dtypes.bfloat16
        )

    core = dst // NSH
    dstl = (dst % NSH).astype(np.int64)
    pos = psrc[src]
    quarter = pos // QN
    srcl = (pos % QN).astype(np.int64) + 1

    flat = (core.astype(np.int64) * NQ + quarter) * NSH + dstl
    kap = np.bincount(flat, minlength=NCORES * NQ * NSH).reshape(NCORES, NQ, NSH)

    layouts = []
    for q in range(NQ):
        # last quarter: big-k first so its trailing chunk is reduce-light and
        # the final perm-gather (and the finalize chain) starts sooner
        descr, cols, kbase, sizes = _ladder_layout(
            kap[:, q, :], 2, kdesc="balanced", bias=(0.7 if q == NQ - 1 else 0.30)
        )
        offs = np.concatenate([[0], np.cumsum(sizes)]).astype(np.int64)
        layouts.append((descr, cols, kbase, sizes, offs))
    SQ = [int(l[4][-1]) for l in layouts]
    PQ = max(_pad16(max(l[1] for l in layouts)), NP_ + 16)
    G0 = max(max(l[3]) for l in layouts)

    order = np.lexsort((dstl, quarter, core))
    so, do_, qo, co = srcl[order], dstl[order], quarter[order], core[order]

    # combined per-quarter index stream: [SQ[q] slot idxs | NP_ perm idxs]
    qoff = np.concatenate([[0], np.cumsum([s + NP_ for s in SQ])]).astype(np.int64)
    qbase = np.concatenate([[0], np.cumsum(SQ)]).astype(np.int64)
    eidx = np.zeros((NCORES, int(qoff[-1])), dtype=np.int16)
    for c in range(NCORES):
        mc = co == c
        for q in range(NQ):
            m = mc & (qo == q)
            descr, cols, kbase, sizes, offs = layouts[q]
            eslots, ev, pm = _pack_slots(
                kap[c, q], so[m], do_[m], descr, kbase, cols, offs
            )
            eidx[c, qoff[q] + eslots] = ev.astype(np.int16)
            eidx[c, qoff[q] + SQ[q] : qoff[q] + SQ[q] + NSH] = pm

    eidx_w = np.zeros((NCORES, P, int(qoff[-1]) // 16), dtype=np.int16)
    for c in range(NCORES):
        eidx_w[c] = np.tile(_wrap16(eidx[c]), (8, 1))

    dinvb = np.zeros((NCORES, P, NP_), dtype=ml_dtypes.bfloat16)
    dinvsp = np.zeros((NCORES, SPP, SPW), dtype=np.float32)
    for c in range(NCORES):
        dv = dinv[c * NSH : (c + 1) * NSH]
        dinvb[c, :, :NSH] = np.tile(dv.astype(ml_dtypes.bfloat16)[None, :], (P, 1))
        dinvsp[c] = dv.reshape(SPP, SPW)

    w2rep = np.tile(np.asarray(W2, dtype=ml_dtypes.bfloat16).reshape(P, 1), (1, P))

    meta = dict(layouts=layouts, SQ=SQ, PQ=PQ, G0=G0, qbase=qbase, qoff=qoff)
    k1_inputs = []
    for c in range(NCORES):
        k1_inputs.append(
            {
                "xt": xt,
                "w1": np.asarray(W1, dtype=ml_dtypes.bfloat16),
                "b1": np.asarray(b1, dtype=np.float32).reshape(P, 1),
                "w2rep": w2rep,
                "eidx": np.ascontiguousarray(eidx_w[c]),
                "dinvb": np.ascontiguousarray(dinvb[c]),
                "dinvsp": np.ascontiguousarray(dinvsp[c]),
            }
        )
    return k1_inputs, meta, (src, dst, dinv)


def host_prep_k2(src, dst):
    """Layer 2, src-side: each core owns the edges whose SRC is in its node
    range; the 8 GPSIMD groups = 8 DST core ranges. Self-loops excluded."""
    m = src != dst
    s2, d2 = src[m].astype(np.int64), dst[m].astype(np.int64)
    c2 = s2 // NSH  # owning core (src side)
    g2 = d2 // NSH  # GPSIMD group (dst core)
    dstl = d2 % NSH
    srcl = s2 % NSH + 1  # index into own z table [zero col + 6250]

    flat = (c2 * NCORES + g2) * NSH + dstl
    kap2 = np.bincount(flat, minlength=NCORES * NCORES * NSH).reshape(
        NCORES * NCORES, NSH
    )
    descr2, cols2, kbase2, sizes2 = _ladder_layout(kap2, 2, kdesc="balanced", bias=0.55)
    offs2 = np.concatenate([[0], np.cumsum(sizes2)]).astype(np.int64)
    slots2 = int(offs2[-1])
    P2 = _pad16(cols2)

    order = np.lexsort((dstl, g2, c2))
    so, do_, go, co = srcl[order], dstl[order], g2[order], c2[order]
    eidx2 = np.zeros((NCORES, NCORES, slots2), dtype=np.int16)
    perm2 = np.zeros((NCORES, NCORES, NP_), dtype=np.int16)
    for c in range(NCORES):
        mc = co == c
        for g in range(NCORES):
            mm = mc & (go == g)
            eslots, ev, pm = _pack_slots(
                kap2[c * NCORES + g], so[mm], do_[mm], descr2, kbase2, cols2, offs2
            )
            eidx2[c, g, eslots] = ev.astype(np.int16)
            perm2[c, g, :NSH] = pm

    eidx2_w = np.zeros((NCORES, P, slots2 // 16), dtype=np.int16)
    perm2_w = np.zeros((NCORES, P, NP_ // 16), dtype=np.int16)
    for c in range(NCORES):
        for g in range(NCORES):
            eidx2_w[c, g * 16 : (g + 1) * 16] = _wrap16(eidx2[c, g])
            perm2_w[c, g * 16 : (g + 1) * 16] = _wrap16(perm2[c, g])

    meta2 = dict(descr2=descr2, P2=P2, slots2=slots2, sizes2=sizes2, offs2=offs2)
    k2_inputs = []
    for c in range(NCORES):
        k2_inputs.append(
            {
                "eidx2": np.ascontiguousarray(eidx2_w[c]),
                "perm2": np.ascontiguousarray(perm2_w[c]),
            }
        )
    return k2_inputs, meta2


def build_fused(meta, meta2, b2val):
    layouts, SQ, PQ, G0, qbase, qoff = (
        meta["layouts"],
        meta["SQ"],
        meta["PQ"],
        meta["G0"],
        meta["qbase"],
        meta["qoff"],
    )
    ITW = (max(SQ) + NP_) // 16  # combined per-quarter idx tile width
    descr2, P2, slots2 = meta2["descr2"], meta2["P2"], meta2["slots2"]
    sizes2, offs2 = meta2["sizes2"], meta2["offs2"]
    G2 = max(sizes2)

    nc = bacc.Bacc(None, target_bir_lowering=False)
    f32, bf16, i16 = (mybir.dt.float32, mybir.dt.bfloat16, mybir.dt.int16)

    xt_d = nc.dram_tensor("xt", [P, NQ * T], bf16, kind="ExternalInput")
    w1_d = nc.dram_tensor("w1", [P, P], bf16, kind="ExternalInput")
    b1_d = nc.dram_tensor("b1", [P, 1], f32, kind="ExternalInput")
    w2rep_d = nc.dram_tensor("w2rep", [P, P], bf16, kind="ExternalInput")
    eidx_d = nc.dram_tensor("eidx", [P, int(qoff[-1]) // 16], i16, kind="ExternalInput")
    dinvb_d = nc.dram_tensor("dinvb", [P, NP_], bf16, kind="ExternalInput")
    dinvsp_d = nc.dram_tensor("dinvsp", [SPP, SPW], f32, kind="ExternalInput")
    eidx2_d = nc.dram_tensor("eidx2", [P, slots2 // 16], i16, kind="ExternalInput")
    perm2_d = nc.dram_tensor("perm2", [P, NP_ // 16], i16, kind="ExternalInput")
    out_d = nc.dram_tensor("out", [SPP, SPW], f32, kind="ExternalOutput")

    zrow_d = nc.dram_tensor("zrow_cc", [1, NSH], f32, kind="Internal")
    zpin = nc.dram_tensor("zpin_cc", [NCORES, NSH], f32, kind="Internal")
    zpout = nc.dram_tensor("zpout_cc", [1, NSH], f32, kind="Internal")

    Copy = mybir.ActivationFunctionType.Copy
    Sigmoid = mybir.ActivationFunctionType.Sigmoid
    ADD = mybir.AluOpType.add

    seg_pe = {"pool": None, "ident": None}

    def seg_sum(accp, g, off, n_rows, k, col):
        # k-segment sum. Big k=2/3 classes go to the idle PE as accumulating
        # identity matmuls over strided column views (psum-chunked to one
        # bank), with an Act copy back; small k on DVE as (k-1) strided adds
        # (DVE is charged max operand free-size, so each add costs n_rows
        # instead of reduce's k*n_rows); large k keeps the single reduce.
        lanes = g[:, off : off + n_rows * k].rearrange("p (a b) -> p a b", a=n_rows, b=k)
        if False and seg_pe["pool"] is not None and k <= 3 and n_rows >= 256:
            pp = seg_pe["pool"]
            idt = seg_pe["ident"]
            for c0 in range(0, n_rows, MMCH):
                cw = min(MMCH, n_rows - c0)
                ps = pp.tile([P, MMCH], f32, tag="pair")
                for j in range(k):
                    nc.tensor.matmul(
                        ps[:, :cw],
                        idt[:],
                        lanes[:, c0 : c0 + cw, j],
                        start=(j == 0),
                        stop=(j == k - 1),
                    )
                nc.scalar.activation(
                    accp[:, col + c0 : col + c0 + cw], ps[:, :cw], Copy
                )
            return
        if k > 5 or (k >= 3 and n_rows < 96):
            nc.vector.tensor_reduce(
                accp[:, col : col + n_rows],
                g[:, off : off + n_rows * k].rearrange(
                    "p (a b) -> p a b", a=n_rows, b=k
                ),
                axis=mybir.AxisListType.X,
                op=ADD,
            )
            return
        lanes = g[:, off : off + n_rows * k].rearrange("p (a b) -> p a b", a=n_rows, b=k)
        nc.vector.tensor_add(
            accp[:, col : col + n_rows], lanes[:, :, 0], lanes[:, :, 1]
        )
        for j in range(2, k):
            nc.vector.tensor_add(
                accp[:, col : col + n_rows],
                accp[:, col : col + n_rows],
                lanes[:, :, j],
            )

    with ExitStack() as ctx:
        tc = ctx.enter_context(TileContext(nc))
        cpool = ctx.enter_context(tc.tile_pool(name="cpool", bufs=1))
        w1 = cpool.tile([P, P], bf16)
        b1 = cpool.tile([P, 1], f32)
        w2rep = cpool.tile([P, P], bf16)
        acc = cpool.tile([P, NP_], bf16)
        warm = cpool.tile([1, 16], f32)
        # b1/w2rep are finalize-only: loaded late so they don't delay tab 0
        # preload the sigmoid activation table off the critical path
        nc.vector.memset(warm[:], 0.0)
        nc.scalar.activation(warm[:], warm[:], Sigmoid, bias=0.0)

        with (
            tc.tile_pool(name="accpool", bufs=2) as accpool,
            tc.tile_pool(name="epool", bufs=2) as epool,
        ):
            accps = {}

            gctx = ExitStack()
            gpool = gctx.enter_context(tc.tile_pool(name="gpool", bufs=2))
            with (
                tc.tile_pool(name="tabs", bufs=2) as tabs,
                tc.tile_pool(name="xpool", bufs=2) as xpool,
                tc.tile_pool(name="tpool", bufs=1) as tpool,
                tc.tile_pool(name="pspool", bufs=2, space="PSUM") as pspool,
            ):

                tts = {}

                def assemble_pool(q):
                    # perm-gather quarter q's partials to node order (emitted
                    # one quarter late, after the next quarter's 2nd gather)
                    accp_q, it_q = accps.pop(q)
                    p0 = SQ[q] // 16
                    tt = gpool.tile([P, G0], f32, tag="g")
                    nc.gpsimd.ap_gather(
                        tt[:, :NP_],
                        accp_q[:],
                        it_q[:, p0 : p0 + NP_ // 16],
                        channels=P,
                        num_elems=PQ,
                        d=1,
                        num_idxs=NP_,
                    )
                    tts[q] = tt

                tbp = {}

                def assemble_convert(q):
                    tt = tts.pop(q)
                    tb = tpool.tile([P, NP_], bf16, tag="tb")
                    nc.scalar.activation(tb[:], tt[:, :NP_], Copy)
                    tbp[q] = tb

                def assemble_add(q):
                    nc.vector.tensor_add(acc[:], acc[:], tbp.pop(q)[:])

                def assemble_fold(q):
                    # fold tt into acc; emitted after the current chunk's k1
                    # copies so they don't queue behind this convert on Act
                    if q == 0:
                        tt = tts.pop(q)
                        nc.scalar.activation(acc[:], tt[:, :NP_], Copy)
                    else:
                        assemble_convert(q)
                        assemble_add(q)

                tabq = {}

                def build_tab(q, first=False):
                    # software pipeline: tab q is built (DMA + MM + Act copy)
                    # at the start of quarter q-1 so its Act copies precede
                    # that quarter's k1/assemble work in the Act queue
                    tab = tabs.tile([P, T], f32, tag="tab")
                    it = epool.tile([P, ITW], i16, tag="it")
                    qw = (SQ[q] + NP_) // 16
                    i0 = int(qoff[q]) // 16
                    x0 = 0
                    first_dma = True
                    while x0 < T:
                        xw = min(XB, T - x0)
                        xc = xpool.tile([P, XB], bf16, tag="x")
                        nc.sync.dma_start(
                            out=xc[:, :xw], in_=xt_d[:, q * T + x0 : q * T + x0 + xw]
                        )
                        if first and first_dma:
                            nc.sync.dma_start(out=w1[:], in_=w1_d[:])
                        if first_dma:
                            nc.sync.dma_start(
                                out=it[:, :qw], in_=eidx_d[:, i0 : i0 + qw]
                            )
                            first_dma = False
                        for p0 in range(0, xw, 1024):
                            pw = min(1024, xw - p0)
                            ps = pspool.tile([P, 1024], f32, tag="ps")
                            for m0 in range(p0, p0 + pw, MMCH):
                                mw = min(MMCH, p0 + pw - m0)
                                nc.tensor.matmul(
                                    ps[:, m0 - p0 : m0 - p0 + mw],
                                    w1[:],
                                    xc[:, m0 : m0 + mw],
                                    start=True,
                                    stop=True,
                                )
                            nc.scalar.activation(
                                tab[:, x0 + p0 : x0 + p0 + pw], ps[:, :pw], Copy
                            )
                        x0 += xw
                    tabq[q] = (tab, it)

                build_tab(0, first=True)
                for q in range(NQ):
                    if q + 1 < NQ:
                        build_tab(q + 1)
                    descr, cols, kbase, sizes, offs = layouts[q]
                    tab, it = tabq.pop(q)
                    accp = accpool.tile([P, PQ], f32, tag="accp")
                    nc.vector.memset(accp[:, 0:1], 0.0)
                    accps[q] = (accp, it)
                    by_chunk = {}
                    for d_ in descr:
                        by_chunk.setdefault(d_[0], []).append(d_)
                    for ci, ch in enumerate(sorted(by_chunk)):
                        sz = sizes[ch]
                        c0 = int(offs[ch]) // 16
                        g = gpool.tile([P, G0], f32, tag="g")
                        nc.gpsimd.ap_gather(
                            g[:, :sz],
                            tab[:],
                            it[:, c0 : c0 + sz // 16],
                            channels=P,
                            num_elems=T,
                            d=1,
                            num_idxs=sz,
                        )
                        if ci == 1 and q > 0 and (q - 1) in accps:
                            assemble_pool(q - 1)
                        for _, off, n_rows, k, col in by_chunk[ch]:
                            if k == 1:
                                nc.scalar.activation(
                                    accp[:, col : col + n_rows],
                                    g[:, off : off + n_rows],
                                    Copy,
                                )
                            elif (
                                q == NQ - 1 and ci == 1 and k == 2 and n_rows >= 256
                            ):
                                # Pool idles between the last gather and the
                                # tail halves: absorb the big k=2 class there
                                lanes = g[:, off : off + n_rows * 2].rearrange(
                                    "p (a b) -> p a b", a=n_rows, b=2
                                )
                                nc.gpsimd.tensor_add(
                                    accp[:, col : col + n_rows],
                                    lanes[:, :, 0],
                                    lanes[:, :, 1],
                                )
                            else:
                                seg_sum(accp, g, off, n_rows, k, col)
                    if (q - 1) in tts:
                        assemble_fold(q - 1)
                    if q == NQ - 2:
                        # zero-lag assembly for the 2nd-to-last quarter: the
                        # perm and bf16 convert run under the last quarter;
                        # the acc add is deferred into the tail (it would
                        # otherwise sit ahead of the last quarter's reduces
                        # in the DVE queue)
                        assemble_pool(q)
                        assemble_convert(q)

            seg_pe["pool"] = None
            # ---- tail: last assembly interleaved with finalize; then L2 ----
            if True:
                # zt reuses an accp slot (freed by the zero-lag q5 assembly);
                # eidx2/perm2/spd ride epool slots so no new pool is needed
                # and gpool can close before the L2 pools open (stack order)
                zt = accpool.tile([P, NP_ + 16], f32, tag="accp")
                eidx2 = epool.tile([P, ITW], i16, tag="it")
                perm2 = epool.tile([P, ITW], i16, tag="it")
                spd = epool.tile([SPP, SPW], f32, tag="it")
                nc.vector.memset(zt[:, 0:1], 0.0)
                nc.scalar.dma_start(out=b1[:], in_=b1_d[:])
                nc.scalar.dma_start(out=w2rep[:], in_=w2rep_d[:])
                nc.sync.dma_start(out=eidx2[:, : slots2 // 16], in_=eidx2_d[:])
                nc.sync.dma_start(out=perm2[:, : NP_ // 16], in_=perm2_d[:])
                nc.sync.dma_start(out=spd[:], in_=dinvsp_d[:])

                with (
                    tc.tile_pool(name="finb", bufs=1) as finb,
                    tc.tile_pool(name="tbs", bufs=2) as tbs,
                    tc.tile_pool(name="zps", bufs=2, space="PSUM") as zps,
                ):
                    dinvb = finb.tile([P, NP_], bf16)
                    HB = NP_ // 2
                    nc.sync.dma_start(out=dinvb[:, :HB], in_=dinvb_d[:, :HB])
                    nc.sync.dma_start(out=dinvb[:, HB:], in_=dinvb_d[:, HB:])

                    def finalize_slice(a, b, flip):
                        # h = sigmoid(dinv*acc+b1); z = dinv * (W2^T h), z
                        # broadcast across partitions via replicated W2.
                        # Alternate the dinv fold between DVE (psum mul) and
                        # Act (copy after a cheap bf16 premul) to balance.
                        nc.vector.tensor_mul(
                            acc[:, a:b], acc[:, a:b], dinvb[:, a:b]
                        )
                        nc.scalar.activation(
                            acc[:, a:b], acc[:, a:b], Sigmoid, bias=b1[:, 0:1]
                        )
                        if flip:
                            nc.vector.tensor_mul(
                                acc[:, a:b], acc[:, a:b], dinvb[:, a:b]
                            )
                        for m0 in range(a, b, MMCH):
                            mw = min(MMCH, b - m0)
                            ps = zps.tile([P, MMCH], f32, tag="zp")
                            nc.tensor.matmul(
                                ps[:, :mw],
                                w2rep[:],
                                acc[:, m0 : m0 + mw],
                                start=True,
                                stop=True,
                            )
                            if flip:
                                nc.scalar.activation(
                                    zt[:, 1 + m0 : 1 + m0 + mw], ps[:, :mw], Copy
                                )
                            else:
                                nc.vector.tensor_mul(
                                    zt[:, 1 + m0 : 1 + m0 + mw],
                                    ps[:, :mw],
                                    dinvb[:, m0 : m0 + mw],
                                )

                    # last quarter: perm-gather in 2 halves, assembly and
                    # finalize column-sliced so z production starts early
                    assemble_add(NQ - 2)
                    accp_q, it_q = accps.pop(NQ - 1)
                    p0 = SQ[NQ - 1] // 16
                    for h0, h1 in ((0, 3136), (3136, NP_)):
                        hw_ = h1 - h0
                        tt = finb.tile([P, 3136], f32, tag=f"tt{h0}")
                        nc.gpsimd.ap_gather(
                            tt[:, :hw_],
                            accp_q[:],
                            it_q[:, p0 + h0 // 16 : p0 + h1 // 16],
                            channels=P,
                            num_elems=PQ,
                            d=1,
                            num_idxs=hw_,
                        )
                        step = hw_ // 4
                        for s in range(4):
                            a = h0 + s * step
                            b = a + step
                            with nc.allow_low_precision(reason="bf16 acc"):
                                nc.vector.tensor_add(
                                    acc[:, a:b], acc[:, a:b], tt[:, a - h0 : b - h0]
                                )
                            finalize_slice(a, b, flip=(s % 2 == 1))

                    nc.sync.dma_start(out=zrow_d[:], in_=zt[0:1, 1 : 1 + NSH])

                gctx.close()  # free gather-pool space for the L2 pools
                # ---- layer 2 (src-side) ----
                with (
                    tc.tile_pool(name="l2a", bufs=1) as l2a,
                    tc.tile_pool(name="g2pool", bufs=2) as g2pool,
                    tc.tile_pool(name="zps2", bufs=2, space="PSUM") as zps2,
                ):
                    seg_pe["pool"] = zps2
                    accp2 = l2a.tile([P, P2], f32)
                    nc.vector.memset(accp2[:, 0:1], 0.0)
                    by_chunk2 = {}
                    for d_ in descr2:
                        by_chunk2.setdefault(d_[0], []).append(d_)
                    for ch in sorted(by_chunk2):
                        sz = sizes2[ch]
                        g2 = g2pool.tile([P, G2], f32, tag="g2")
                        i0 = int(offs2[ch]) // 16
                        nc.gpsimd.ap_gather(
                            g2[:, :sz],
                            zt[:, :NP_],
                            eidx2[:, i0 : i0 + sz // 16],
                            channels=P,
                            num_elems=NP_,
                            d=1,
                            num_idxs=sz,
                        )
                        for _, off, n_rows, k, col in by_chunk2[ch]:
                            if k == 1:
                                nc.scalar.activation(
                                    accp2[:, col : col + n_rows],
                                    g2[:, off : off + n_rows],
                                    Copy,
                                )
                            else:
                                seg_sum(accp2, g2, off, n_rows, k, col)
                    g2p = l2a.tile([P, NP_], f32)
                    nc.gpsimd.ap_gather(
                        g2p[:],
                        accp2[:],
                        perm2[:, : NP_ // 16],
                        channels=P,
                        num_elems=P2,
                        d=1,
                        num_idxs=NP_,
                    )
                    # per-group partial rows -> DRAM (1-D out APs are cheap);
                    # alternate DGE queues so the stores pipeline
                    queues = [nc.sync, nc.scalar]
                    for j in range(NCORES):
                        queues[j % 2].dma_start(
                            out=zpin[j : j + 1, :], in_=g2p[16 * j : 16 * j + 1, :NSH]
                        )
                    nc.gpsimd.collective_compute(
                        "ReduceScatter",
                        ADD,
                        replica_groups=[list(range(NCORES))],
                        ins=[zpin[:].opt()],
                        outs=[zpout[:].opt()],
                    )
                    # final: out = sigmoid(dinv*(rs + z_self) + b2), all in a
                    # [125, 50] spread layout to keep the tail DMAs wide
                    spr = l2a.tile([SPP, SPW], f32)
                    spz = l2a.tile([SPP, SPW], f32)
                    nc.sync.dma_start(
                        out=spz[:], in_=zrow_d[:].rearrange("a (p m) -> (a p) m", p=SPP)
                    )
                    nc.sync.dma_start(
                        out=spr[:], in_=zpout[:].rearrange("a (p m) -> (a p) m", p=SPP)
                    )
                    nc.vector.tensor_add(spr[:], spr[:], spz[:])
                    nc.vector.tensor_mul(spr[:], spr[:], spd[:])
                    nc.scalar.activation(spr[:], spr[:], Sigmoid, bias=float(b2val))
                    nc.sync.dma_start(out=out_d[:], in_=spr[:])
    nc.finalize()
    return nc


def _sim_ns(nc):
    from concourse import bass_interp

    sim = bass_interp.CoreSim(nc, no_exec=True, publish_trace=False)
    sim.simulate()
    return int(sim.time)


def kernel(x, edge_index, W1, b1, W2, b2):
    global LAST_SIM_NS
    x = np.asarray(x, dtype=np.float32)
    edge_index = np.asarray(edge_index)
    k1_inputs, meta, (src, dst, dinv) = host_prep(x, edge_index, W1, b1, W2, b2)
    k2_inputs, meta2 = host_prep_k2(src, dst)
    b2val = float(np.asarray(b2, dtype=np.float32).reshape(-1)[0])
    nc = build_fused(meta, meta2, b2val)
    if MEASURE:
        LAST_SIM_NS = _sim_ns(nc)
    in_maps = [dict(k1_inputs[c], **k2_inputs[c]) for c in range(NCORES)]
    res = run_bass_kernel_spmd(nc, in_maps, list(range(NCORES)))
    out = np.zeros((N, 1), dtype=np.float32)
    for c in range(NCORES):
        out[c * NSH : (c + 1) * NSH, 0] = res.results[c]["out"].reshape(-1)
    return out
